# revision 31
# baseline (speedup 1.0000x reference)
"""Trainium2 Bass kernel for BaseWindowAttention.

Problem (hardcoded): x [2,8,64,64,256] f32, w_qkv [256,768], w_out [256,256],
b_out [256], pos_embedding [15,15], window_size 8, heads 8, dim_head 32.

Strategy:
- Data parallel: 16 (b,l) images over 8 cores -> 2 images/core.
- Host: window-major channel-first bf16 transpose of x; fold softmax scale
  into w_q; precompute exp(bias) 2-window super-tile (off-diagonal zeros kill
  cross-window attention terms); b_out added host-side after the gather.
- Device per core, per strip of 512 tokens (8 windows):
  q/k projection ([o,t] layout), v projection ([t,o] layout, head-strided with
  an appended ones column for the softmax denominator), window-pair dots as
  4 row-group-packed [32,128]x[32,128] matmuls, ACT exp, GpSimd/DVE multiply
  by exp(bias) mask tile, AV matmul (fused denominator), reciprocal +
  broadcast normalize, PE transpose to [hc,t], out-projection, per-wp
  output DMA.
- PSUM banks: qk/v projections share a 2-buf [128,512] tag, dots use two
  2-bank tiles, av/transpose/out-proj share a 2-buf small tag (8 banks
  total).  Split x DMAs ([128,256] quarters) shorten prefetch latency.
  Copies routed via nc.any so the Tile scheduler balances ACT/DVE; the
  exp(bias) mask multiply is split 3 GpSimd / 1 DVE.

Perf notes (measured on trn2 via NTFF): baseline 294us -> this version
~197us.  PE (tensor engine) is the critical resource: LDWEIGHTS fully
overlaps MATMUL, so instruction *time* not load count dominates; HAM
p-state (1.2 vs 2.4 GHz) oscillates with PE idle gaps, so PSUM slot
availability (qkps bufs=2) and deep SBUF pools matter more than per-op
engine choice.  The largest single win after PSUM rebalancing was
splitting each exp(bias) mask-multiply into two wp-pair halves with
separate edm tiles, h0 halves emitted before h1 halves (218 -> 197us):
AV for wp 0-1 then gates on only half the serialized GpSimd mul chain.
EDM_DVE=1 beats 2 or 4 in both structures.  Attempted and rejected:
DMA-XBAR transpose (Sync engine serializes ~1.2us/transpose -> 2x
slower), per-window garbage-free dots via column tile_position packing
(two stationary tiles sharing a PE row band corrupt results: one
window's outputs zero out), fp8 projections (error budget too tight).
"""

import os
import sys
import numpy as np

sys.path.insert(0, "/opt/trn_rl_repo")
os.environ.setdefault("JAX_PLATFORMS", "")

import ml_dtypes

BF16 = ml_dtypes.bfloat16

B, L, H, W, C = 2, 8, 64, 64, 256
WS = 8
NHEADS = 8
CH = 32
N_CORES = 8
IMG = B * L                 # 16 images
IMG_PER_CORE = IMG // N_CORES
T_IMG = H * W               # 4096 tokens per image
STRIP = 512                 # tokens per strip (8 windows)
N_STRIPS = T_IMG // STRIP   # 8
NWP = STRIP // 128          # 4 window pairs per strip

_CACHE = {}


def _relative_indices(ws):
    idx = np.array([[i, j] for i in range(ws) for j in range(ws)])
    rel = idx[None, :, :] - idx[:, None, :] + ws - 1
    return rel


def _build_kernel():
    import concourse.bass as bass
    import concourse.mybir as mybir
    import concourse.tile as tile
    from concourse import bacc

    dt = mybir.dt
    nc = bacc.Bacc("TRN2", target_bir_lowering=False, debug=False)

    xT = nc.dram_tensor("xT", [IMG_PER_CORE, C, T_IMG], dt.bfloat16,
                        kind="ExternalInput").ap()
    wqk = nc.dram_tensor("wqk", [C, 512], dt.bfloat16, kind="ExternalInput").ap()
    wv = nc.dram_tensor("wv", [C, C], dt.bfloat16, kind="ExternalInput").ap()
    wout = nc.dram_tensor("wout", [C, C], dt.bfloat16, kind="ExternalInput").ap()
    ebrep = nc.dram_tensor("ebrep", [128, 2048], dt.bfloat16,
                           kind="ExternalInput").ap()
    ident = nc.dram_tensor("ident", [128, 128], dt.bfloat16,
                           kind="ExternalInput").ap()
    out = nc.dram_tensor("out", [IMG_PER_CORE, T_IMG, C], dt.bfloat16,
                         kind="ExternalOutput").ap()

    EXP = mybir.ActivationFunctionType.Exp

    with tile.TileContext(nc) as tc:
        from contextlib import ExitStack
        with ExitStack() as ctx:
            consts = ctx.enter_context(tc.tile_pool(name="consts", bufs=1))
            xp = ctx.enter_context(tc.tile_pool(name="xp", bufs=3))
            qkp = ctx.enter_context(tc.tile_pool(name="qkp", bufs=8))
            vp = ctx.enter_context(tc.tile_pool(name="vp", bufs=8))
            ep = ctx.enter_context(tc.tile_pool(name="ep", bufs=3))
            anp = ctx.enter_context(tc.tile_pool(name="anp", bufs=3))
            aotp = ctx.enter_context(tc.tile_pool(name="aotp", bufs=4))
            rdp = ctx.enter_context(tc.tile_pool(name="rdp", bufs=3))
            fop = ctx.enter_context(tc.tile_pool(name="fop", bufs=3))
            psp = ctx.enter_context(tc.tile_pool(name="psp", bufs=1, space="PSUM"))

            # ---- strip-0 inputs first (head latency), then constants.
            # Big constants are split so no single DMA engine serializes
            # a large transfer in front of the first matmuls.
            pre_x = []
            for half in range(2):
                xt = xp.tile([128, STRIP], dt.bfloat16,
                             tag=("xa" if half == 0 else "xb"), bufs=6)
                for q in range(2):
                    nc.sync.dma_start(
                        out=xt[:, q * 256:(q + 1) * 256],
                        in_=xT[0, half * 128:half * 128 + 128,
                               q * 256:(q + 1) * 256])
                pre_x.append(xt)

            wqk_sb = []
            wv_sb = []
            wout_sb = []
            for kk in range(2):
                wqk_t = consts.tile([128, 512], dt.bfloat16, tag=f"wqk{kk}")
                for q in range(2):
                    nc.sync.dma_start(
                        out=wqk_t[:, q * 256:(q + 1) * 256],
                        in_=wqk[kk * 128:(kk + 1) * 128,
                                q * 256:(q + 1) * 256])
                wqk_sb.append(wqk_t)
                wv_t = consts.tile([128, 256], dt.bfloat16, tag=f"wv{kk}")
                nc.sync.dma_start(out=wv_t, in_=wv[kk * 128:(kk + 1) * 128, :])
                wv_sb.append(wv_t)
                wout_t = consts.tile([128, 256], dt.bfloat16, tag=f"wout{kk}")
                nc.sync.dma_start(out=wout_t, in_=wout[kk * 128:(kk + 1) * 128, :])
                wout_sb.append(wout_t)
            eb_sb = consts.tile([128, 4, 512], dt.bfloat16, tag="eb")
            ebr = ebrep.rearrange("p (r c) -> p r c", r=4)
            for r in range(4):
                nc.sync.dma_start(out=eb_sb[:, r, :], in_=ebr[:, r, :])
            id_sb = consts.tile([128, 128], dt.bfloat16, tag="id")
            nc.sync.dma_start(out=id_sb, in_=ident)

            QKPS_BUFS = int(os.environ.get("QKPS_BUFS", "2"))
            SMALLPS_BUFS = int(os.environ.get("SMALLPS_BUFS", "2"))
            DPS_BUFS = int(os.environ.get("DPS_BUFS", "2"))
            EDM_DVE = int(os.environ.get("EDM_DVE", "1"))  # groups on DVE (of 4)
            V3_BUFS = 16
            # ones columns of the v3 ring are written once here; the loop
            # only ever writes [:, :, 0:CH], so they persist across reuse
            for _ in range(V3_BUFS):
                v3i = vp.tile([128, NHEADS, CH + 1], dt.bfloat16,
                              tag="v3", bufs=V3_BUFS)
                nc.gpsimd.memset(v3i[:, :, CH:CH + 1], 1.0)

            orr = out.rearrange("i (s w p) c -> i s p w c",
                                s=N_STRIPS, w=NWP)

            # ---- software pipeline: the per-wp tail chain of strip s-1
            # (AV -> normalize -> transpose -> out-proj -> store) ping-pongs
            # PE<->DVE with no slack, and the in-order PE queue head-of-line
            # blocks on each hop.  Split each tail into three sub-phases and
            # interleave them between the front-phase matmul groups of strip
            # s, so every dependent PE op reaches the queue head with its
            # cross-engine producer long finished.
            def tail_av(st, wp):
                """AV matmuls (+denominator col) and normalize for one wp."""
                edm_sb, v_sb, fo, img_, s_, attn_sb, aot_sbs = st
                cc = (wp % 2) * 128
                avps = psp.tile([128, NHEADS, CH + 1], dt.float32,
                                tag="smallps", bufs=SMALLPS_BUFS)
                for hg in range(2):
                    for rg in range(4):
                        h = 4 * hg + rg
                        nc.tensor.matmul(
                            avps[:, h, :],
                            edm_sb[(hg, rg // 2, wp // 2)][:, rg % 2,
                                                           cc:cc + 128],
                            v_sb[wp][:, h, :],
                            start=True, stop=True,
                        )
                rd = rdp.tile([128, NHEADS, 1], dt.float32, tag="rd", bufs=8)
                nc.vector.reciprocal(rd, avps[:, :, CH:CH + 1])
                attn = anp.tile([128, NHEADS, CH], dt.bfloat16,
                                tag="attn", bufs=8)
                nc.vector.tensor_mul(attn, avps[:, :, 0:CH],
                                     rd.to_broadcast((128, NHEADS, CH)))
                attn_sb[wp] = attn

            def tail_tp(st, wp):
                """transpose [t,hc] -> [hc,t] + evacuate for one wp."""
                edm_sb, v_sb, fo, img_, s_, attn_sb, aot_sbs = st
                attn = attn_sb[wp]
                tps = psp.tile([128, 2, 128], dt.bfloat16,
                               tag="smallps", bufs=SMALLPS_BUFS)
                for half in range(2):
                    nc.tensor.transpose(
                        tps[:, half, :],
                        attn[:, half * 4:(half + 1) * 4, :], id_sb)
                aot = aotp.tile([128, 2, 128], dt.bfloat16, tag="aot", bufs=8)
                nc.any.tensor_copy(aot, tps)
                aot_sbs[wp] = aot

            def tail_op(st, wp):
                """out-projection + store copy for one wp (store DMA on
                the last wp)."""
                edm_sb, v_sb, fo, img_, s_, attn_sb, aot_sbs = st
                aot = aot_sbs[wp]
                ops = psp.tile([128, 256], dt.float32, tag="smallps",
                               bufs=SMALLPS_BUFS)
                nc.tensor.matmul(ops, aot[:, 0, :], wout_sb[0],
                                 start=True, stop=False)
                nc.tensor.matmul(ops, aot[:, 1, :], wout_sb[1],
                                 start=False, stop=True)
                nc.any.tensor_copy(fo[:, wp, :], ops)
                if wp == NWP - 1:
                    nc.sync.dma_start(out=orr[img_, s_], in_=fo)

            PIPE_LAG = int(os.environ.get("PIPE_LAG", "1"))
            pending = []   # oldest-first states awaiting tails
            prev = None
            strips = [(img, s) for img in range(IMG_PER_CORE)
                      for s in range(N_STRIPS)]
            for img, s in strips:
                    prev = pending.pop(0) if len(pending) >= PIPE_LAG else None
                    t0 = s * STRIP
                    if img == 0 and s == 0:
                        xa, xb = pre_x
                    else:
                        xa = xp.tile([128, STRIP], dt.bfloat16, tag="xa", bufs=6)
                        nc.sync.dma_start(out=xa,
                                          in_=xT[img, 0:128, t0:t0 + 512])
                        xb = xp.tile([128, STRIP], dt.bfloat16, tag="xb", bufs=6)
                        nc.sync.dma_start(out=xb,
                                          in_=xT[img, 128:256, t0:t0 + 512])

                    # ---- q/k projection: out [o=128 (4 heads), t=512]
                    # order q03, k03, q47, k47: head-group 0's dots need
                    # only the first two copies
                    qk_sb = [None] * 4
                    for i, ot in enumerate((0, 2, 1, 3)):
                        qkps = psp.tile([128, STRIP], dt.float32, tag="qkps",
                                        bufs=QKPS_BUFS)
                        nc.tensor.matmul(qkps, wqk_sb[0][:, ot * 128:(ot + 1) * 128],
                                         xa, start=True, stop=False)
                        nc.tensor.matmul(qkps, wqk_sb[1][:, ot * 128:(ot + 1) * 128],
                                         xb, start=False, stop=True)
                        qk_t = qkp.tile([128, STRIP], dt.bfloat16, tag="qk_t", bufs=12)
                        nc.any.tensor_copy(qk_t, qkps)
                        qk_sb[ot] = qk_t
                        if prev is not None:
                            if i == 1:
                                tail_av(prev, 0)
                            elif i == 2:
                                tail_av(prev, 1)
                            elif i == 3:
                                tail_tp(prev, 0)

                    # ---- dots -> exp; mask-mult per (hg, half)
                    edm_sb = {}
                    ed_sb = {}
                    for j, (hg, half) in enumerate(
                            ((0, 0), (0, 1), (1, 0), (1, 1))):
                            dps = psp.tile([128, 2, 512], dt.float32,
                                           tag="dps", bufs=DPS_BUFS)
                            for wp in range(NWP):
                                c0 = wp * 128
                                for r2 in range(2):
                                    rg = 2 * half + r2
                                    nc.tensor.matmul(
                                        dps[:, r2, c0:c0 + 128],
                                        qk_sb[2 + hg][32 * rg:32 * rg + 32,
                                                      c0:c0 + 128],
                                        qk_sb[hg][32 * rg:32 * rg + 32,
                                                  c0:c0 + 128],
                                        start=True, stop=True,
                                        tile_position=(32 * rg, 0),
                                    )
                            ed = ep.tile([128, 2, 512], dt.bfloat16, tag="ed", bufs=10)
                            nc.scalar.activation(ed, dps, EXP)
                            ed_sb[(hg, half)] = ed
                            if prev is not None:
                                if j == 0:
                                    tail_tp(prev, 1)
                                elif j == 1:
                                    tail_op(prev, 0)
                                    tail_av(prev, 2)
                                elif j == 2:
                                    tail_tp(prev, 2)
                                    tail_op(prev, 1)
                                elif j == 3:
                                    tail_av(prev, 3)
                    for wh in range(2):
                        cs = slice(wh * 256, wh * 256 + 256)
                        for hg in range(2):
                            for half in range(2):
                                edm = ep.tile([128, 2, 256], dt.bfloat16,
                                              tag="edm", bufs=32)
                                gi = 2 * hg + half
                                if gi >= 4 - EDM_DVE:
                                    nc.vector.tensor_mul(
                                        edm, ed_sb[(hg, half)][:, :, cs],
                                        eb_sb[:, 2 * half:2 * half + 2, cs])
                                else:
                                    nc.gpsimd.tensor_mul(
                                        edm, ed_sb[(hg, half)][:, :, cs],
                                        eb_sb[:, 2 * half:2 * half + 2, cs])
                                edm_sb[(hg, half, wh)] = edm
                    if prev is not None:
                        tail_op(prev, 2)
                        tail_tp(prev, 3)

                    # ---- v projection: out [t=128, 8, 32] + ones col
                    v_sb = []
                    for tb in range(NWP):
                        vps = psp.tile([128, NHEADS, CH], dt.float32,
                                       tag="qkps", bufs=QKPS_BUFS)
                        nc.tensor.matmul(vps, xa[:, tb * 128:(tb + 1) * 128],
                                         wv_sb[0], start=True, stop=False)
                        nc.tensor.matmul(vps, xb[:, tb * 128:(tb + 1) * 128],
                                         wv_sb[1], start=False, stop=True)
                        v3 = vp.tile([128, NHEADS, CH + 1], dt.bfloat16,
                                     tag="v3", bufs=V3_BUFS)
                        nc.any.tensor_copy(v3[:, :, 0:CH], vps)
                        v_sb.append(v3)
                        if prev is not None and tb == 1:
                            tail_op(prev, 3)

                    fo = fop.tile([128, NWP, 256], dt.bfloat16, tag="fo",
                                  bufs=4)
                    pending.append((edm_sb, v_sb, fo, img, s, [None] * NWP,
                                    [None] * NWP))
            # drain remaining strips' tails
            for st in pending:
                for wp in range(NWP):
                    tail_av(st, wp)
                    tail_tp(st, wp)
                    tail_op(st, wp)
    nc.compile()
    return nc


def _host_prep(x, w_qkv, w_out, b_out, pos_embedding):
    ws = WS
    scale = CH ** -0.5
    xs = x.reshape(B * L, H // ws, ws, W // ws, ws, C)
    xs = xs.transpose(0, 1, 3, 2, 4, 5).reshape(IMG, T_IMG, C)
    xT = np.ascontiguousarray(xs.transpose(0, 2, 1)).astype(BF16)

    wq = (w_qkv[:, 0:256] * scale).astype(BF16)
    wk = w_qkv[:, 256:512].astype(BF16)
    wqk = np.concatenate([wq, wk], axis=1)
    wv = w_qkv[:, 512:768].astype(BF16)

    ri = _relative_indices(ws)
    bias = pos_embedding[ri[:, :, 0], ri[:, :, 1]]  # [i, j]
    ebT = np.exp(bias.astype(np.float64)).T.astype(np.float32)  # [j, i]
    ebsuper = np.zeros((128, 128), np.float32)
    ebsuper[0:64, 0:64] = ebT
    ebsuper[64:128, 64:128] = ebT
    ebrep = np.tile(ebsuper, (1, 16)).astype(BF16)

    ident = np.eye(128, dtype=BF16)

    return {
        "xT": xT,
        "wqk": np.ascontiguousarray(wqk),
        "wv": np.ascontiguousarray(wv),
        "wout": w_out.astype(BF16),
        "ebrep": ebrep,
        "ident": ident,
    }


def kernel(x, w_qkv, w_out, b_out, pos_embedding, window_size, **extra):
    from concourse.bass_utils import run_bass_kernel_spmd

    x = np.asarray(x, dtype=np.float32)
    w_qkv = np.asarray(w_qkv, dtype=np.float32)
    w_out = np.asarray(w_out, dtype=np.float32)
    b_out = np.asarray(b_out, dtype=np.float32)
    pos_embedding = np.asarray(pos_embedding, dtype=np.float32)

    prep = _host_prep(x, w_qkv, w_out, b_out, pos_embedding)

    if "nc" not in _CACHE:
        _CACHE["nc"] = _build_kernel()
    nc = _CACHE["nc"]

    in_maps = []
    for core in range(N_CORES):
        m = dict(prep)
        m["xT"] = np.ascontiguousarray(
            prep["xT"][core * IMG_PER_CORE:(core + 1) * IMG_PER_CORE])
        in_maps.append(m)

    res = run_bass_kernel_spmd(nc, in_maps, core_ids=list(range(N_CORES)))
    outs = [res.results[c]["out"] for c in range(N_CORES)]
    o = np.concatenate(outs, axis=0)  # [16, 4096, 256]
    o = o.reshape(B * L, H // WS, W // WS, WS, WS, C)
    o = o.transpose(0, 1, 3, 2, 4, 5).reshape(B, L, H, W, C)
    o = o.astype(np.float32)
    o += b_out.astype(np.float32)
    return np.ascontiguousarray(o)



# revision 32
# speedup vs baseline: 1.0908x; 1.0908x over previous
"""Trainium2 Bass kernel for BaseWindowAttention.

Problem (hardcoded): x [2,8,64,64,256] f32, w_qkv [256,768], w_out [256,256],
b_out [256], pos_embedding [15,15], window_size 8, heads 8, dim_head 32.

Strategy:
- Data parallel: 16 (b,l) images over 8 cores -> 2 images/core.
- Host: window-major channel-first bf16 transpose of x; fold softmax scale
  into w_q; precompute exp(bias) 2-window super-tile (off-diagonal zeros kill
  cross-window attention terms); b_out added host-side after the gather.
- Device per core, per strip of 512 tokens (8 windows):
  q/k projection ([o,t] layout), v projection ([t,o] layout, head-strided with
  an appended ones column for the softmax denominator), window-pair dots as
  4 row-group-packed [32,128]x[32,128] matmuls, ACT exp, GpSimd/DVE multiply
  by exp(bias) mask tile, AV matmul (fused denominator), reciprocal +
  broadcast normalize, PE transpose to [hc,t], out-projection, per-wp
  output DMA.
- PSUM banks: qk/v projections share a 2-buf [128,512] tag, dots use two
  2-bank tiles, av/transpose/out-proj share a 2-buf small tag (8 banks
  total).  Split x DMAs ([128,256] quarters) shorten prefetch latency.
  Copies routed via nc.any so the Tile scheduler balances ACT/DVE; the
  exp(bias) mask multiply is split 3 GpSimd / 1 DVE.

Perf notes (measured on trn2 via NTFF): baseline 294us -> this version
~197us.  PE (tensor engine) is the critical resource: LDWEIGHTS fully
overlaps MATMUL, so instruction *time* not load count dominates; HAM
p-state (1.2 vs 2.4 GHz) oscillates with PE idle gaps, so PSUM slot
availability (qkps bufs=2) and deep SBUF pools matter more than per-op
engine choice.  The largest single win after PSUM rebalancing was
splitting each exp(bias) mask-multiply into two wp-pair halves with
separate edm tiles, h0 halves emitted before h1 halves (218 -> 197us):
AV for wp 0-1 then gates on only half the serialized GpSimd mul chain.
EDM_DVE=1 beats 2 or 4 in both structures.  Attempted and rejected:
DMA-XBAR transpose (Sync engine serializes ~1.2us/transpose -> 2x
slower), per-window garbage-free dots via column tile_position packing
(two stationary tiles sharing a PE row band corrupt results: one
window's outputs zero out), fp8 projections (error budget too tight).
"""

import os
import sys
import numpy as np

sys.path.insert(0, "/opt/trn_rl_repo")
os.environ.setdefault("JAX_PLATFORMS", "")

import ml_dtypes

BF16 = ml_dtypes.bfloat16

B, L, H, W, C = 2, 8, 64, 64, 256
WS = 8
NHEADS = 8
CH = 32
N_CORES = 8
IMG = B * L                 # 16 images
IMG_PER_CORE = IMG // N_CORES
T_IMG = H * W               # 4096 tokens per image
STRIP = 512                 # tokens per strip (8 windows)
N_STRIPS = T_IMG // STRIP   # 8
NWP = STRIP // 128          # 4 window pairs per strip

_CACHE = {}


def _relative_indices(ws):
    idx = np.array([[i, j] for i in range(ws) for j in range(ws)])
    rel = idx[None, :, :] - idx[:, None, :] + ws - 1
    return rel


def _build_kernel():
    import concourse.bass as bass
    import concourse.mybir as mybir
    import concourse.tile as tile
    from concourse import bacc

    dt = mybir.dt
    nc = bacc.Bacc("TRN2", target_bir_lowering=False, debug=False)

    xT = nc.dram_tensor("xT", [IMG_PER_CORE, C, T_IMG], dt.bfloat16,
                        kind="ExternalInput").ap()
    wqk = nc.dram_tensor("wqk", [C, 512], dt.bfloat16, kind="ExternalInput").ap()
    wv = nc.dram_tensor("wv", [C, C], dt.bfloat16, kind="ExternalInput").ap()
    wout = nc.dram_tensor("wout", [C, C], dt.bfloat16, kind="ExternalInput").ap()
    ebrep = nc.dram_tensor("ebrep", [128, 2048], dt.bfloat16,
                           kind="ExternalInput").ap()
    ident = nc.dram_tensor("ident", [128, 128], dt.bfloat16,
                           kind="ExternalInput").ap()
    out = nc.dram_tensor("out", [IMG_PER_CORE, T_IMG, C], dt.bfloat16,
                         kind="ExternalOutput").ap()

    EXP = mybir.ActivationFunctionType.Exp

    with tile.TileContext(nc) as tc:
        from contextlib import ExitStack
        with ExitStack() as ctx:
            consts = ctx.enter_context(tc.tile_pool(name="consts", bufs=1))
            xp = ctx.enter_context(tc.tile_pool(name="xp", bufs=3))
            qkp = ctx.enter_context(tc.tile_pool(name="qkp", bufs=8))
            vp = ctx.enter_context(tc.tile_pool(name="vp", bufs=8))
            ep = ctx.enter_context(tc.tile_pool(name="ep", bufs=3))
            anp = ctx.enter_context(tc.tile_pool(name="anp", bufs=3))
            aotp = ctx.enter_context(tc.tile_pool(name="aotp", bufs=4))
            rdp = ctx.enter_context(tc.tile_pool(name="rdp", bufs=3))
            fop = ctx.enter_context(tc.tile_pool(name="fop", bufs=3))
            psp = ctx.enter_context(tc.tile_pool(name="psp", bufs=1, space="PSUM"))

            # ---- strip-0 inputs first (head latency), then constants.
            # Big constants are split so no single DMA engine serializes
            # a large transfer in front of the first matmuls.
            pre_x = []
            for half in range(2):
                xt = xp.tile([128, STRIP], dt.bfloat16,
                             tag=("xa" if half == 0 else "xb"), bufs=6)
                for q in range(2):
                    nc.sync.dma_start(
                        out=xt[:, q * 256:(q + 1) * 256],
                        in_=xT[0, half * 128:half * 128 + 128,
                               q * 256:(q + 1) * 256])
                pre_x.append(xt)

            wqk_sb = []
            wv_sb = []
            wout_sb = []
            for kk in range(2):
                wqk_t = consts.tile([128, 512], dt.bfloat16, tag=f"wqk{kk}")
                for q in range(2):
                    nc.sync.dma_start(
                        out=wqk_t[:, q * 256:(q + 1) * 256],
                        in_=wqk[kk * 128:(kk + 1) * 128,
                                q * 256:(q + 1) * 256])
                wqk_sb.append(wqk_t)
                wv_t = consts.tile([128, 256], dt.bfloat16, tag=f"wv{kk}")
                nc.sync.dma_start(out=wv_t, in_=wv[kk * 128:(kk + 1) * 128, :])
                wv_sb.append(wv_t)
                wout_t = consts.tile([128, 256], dt.bfloat16, tag=f"wout{kk}")
                nc.sync.dma_start(out=wout_t, in_=wout[kk * 128:(kk + 1) * 128, :])
                wout_sb.append(wout_t)
            eb_sb = consts.tile([128, 4, 512], dt.bfloat16, tag="eb")
            ebr = ebrep.rearrange("p (r c) -> p r c", r=4)
            for r in range(4):
                nc.sync.dma_start(out=eb_sb[:, r, :], in_=ebr[:, r, :])
            id_sb = consts.tile([128, 128], dt.bfloat16, tag="id")
            nc.sync.dma_start(out=id_sb, in_=ident)

            QKPS_BUFS = int(os.environ.get("QKPS_BUFS", "2"))
            SMALLPS_BUFS = int(os.environ.get("SMALLPS_BUFS", "2"))
            DPS_BUFS = int(os.environ.get("DPS_BUFS", "2"))
            EDM_DVE = int(os.environ.get("EDM_DVE", "1"))  # groups on DVE (of 4)
            V3_BUFS = 12
            # ones columns of the v3 ring are written once here; the loop
            # only ever writes [:, :, 0:CH], so they persist across reuse
            for _ in range(V3_BUFS):
                v3i = vp.tile([128, NHEADS, CH + 1], dt.bfloat16,
                              tag="v3", bufs=V3_BUFS)
                nc.gpsimd.memset(v3i[:, :, CH:CH + 1], 1.0)

            orr = out.rearrange("i (s w p) c -> i s p w c",
                                s=N_STRIPS, w=NWP)

            # ---- software pipeline: the per-wp tail chain of strip s-1
            # (AV -> normalize -> transpose -> out-proj -> store) ping-pongs
            # PE<->DVE with no slack, and the in-order PE queue head-of-line
            # blocks on each hop.  Split each tail into three sub-phases and
            # interleave them between the front-phase matmul groups of strip
            # s, so every dependent PE op reaches the queue head with its
            # cross-engine producer long finished.
            def tail_av(st, wp):
                """AV matmuls (+denominator col) and normalize for one wp."""
                edm_sb, v_sb, fo, img_, s_, attn_sb, aot_sbs = st
                cc = (wp % 2) * 128
                avps = psp.tile([128, NHEADS, CH + 1], dt.float32,
                                tag="smallps", bufs=SMALLPS_BUFS)
                for hg in range(2):
                    for rg in range(4):
                        h = 4 * hg + rg
                        nc.tensor.matmul(
                            avps[:, h, :],
                            edm_sb[(hg, rg // 2, wp // 2)][:, rg % 2,
                                                           cc:cc + 128],
                            v_sb[wp][:, h, :],
                            start=True, stop=True,
                        )
                rd = rdp.tile([128, NHEADS, 1], dt.float32, tag="rd", bufs=8)
                nc.vector.reciprocal(rd, avps[:, :, CH:CH + 1])
                attn = anp.tile([128, NHEADS, CH], dt.bfloat16,
                                tag="attn", bufs=8)
                nc.vector.tensor_mul(attn, avps[:, :, 0:CH],
                                     rd.to_broadcast((128, NHEADS, CH)))
                attn_sb[wp] = attn

            def tail_tp(st, wp):
                """transpose [t,hc] -> [hc,t] + evacuate for one wp."""
                edm_sb, v_sb, fo, img_, s_, attn_sb, aot_sbs = st
                attn = attn_sb[wp]
                tps = psp.tile([128, 2, 128], dt.bfloat16,
                               tag="smallps", bufs=SMALLPS_BUFS)
                for half in range(2):
                    nc.tensor.transpose(
                        tps[:, half, :],
                        attn[:, half * 4:(half + 1) * 4, :], id_sb)
                aot = aotp.tile([128, 2, 128], dt.bfloat16, tag="aot", bufs=8)
                nc.any.tensor_copy(aot, tps)
                aot_sbs[wp] = aot

            def tail_op(st, wp):
                """out-projection + store copy for one wp (store DMA on
                the last wp)."""
                edm_sb, v_sb, fo, img_, s_, attn_sb, aot_sbs = st
                aot = aot_sbs[wp]
                ops = psp.tile([128, 256], dt.float32, tag="smallps",
                               bufs=SMALLPS_BUFS)
                nc.tensor.matmul(ops, aot[:, 0, :], wout_sb[0],
                                 start=True, stop=False)
                nc.tensor.matmul(ops, aot[:, 1, :], wout_sb[1],
                                 start=False, stop=True)
                nc.any.tensor_copy(fo[:, wp, :], ops)
                if wp == NWP - 1:
                    nc.sync.dma_start(out=orr[img_, s_], in_=fo)

            PIPE_LAG = int(os.environ.get("PIPE_LAG", "1"))
            pending = []   # oldest-first states awaiting tails
            prev = None
            strips = [(img, s) for img in range(IMG_PER_CORE)
                      for s in range(N_STRIPS)]
            for img, s in strips:
                    prev = pending.pop(0) if len(pending) >= PIPE_LAG else None
                    t0 = s * STRIP
                    if img == 0 and s == 0:
                        xa, xb = pre_x
                    else:
                        xa = xp.tile([128, STRIP], dt.bfloat16, tag="xa", bufs=6)
                        nc.sync.dma_start(out=xa,
                                          in_=xT[img, 0:128, t0:t0 + 512])
                        xb = xp.tile([128, STRIP], dt.bfloat16, tag="xb", bufs=6)
                        nc.sync.dma_start(out=xb,
                                          in_=xT[img, 128:256, t0:t0 + 512])

                    # ---- q/k projection: out [o=128 (4 heads), t=512]
                    # order q03, k03, q47, k47: head-group 0's dots need
                    # only the first two copies
                    qk_sb = [None] * 4
                    for i, ot in enumerate((0, 2, 1, 3)):
                        qkps = psp.tile([128, STRIP], dt.float32, tag="qkps",
                                        bufs=QKPS_BUFS)
                        nc.tensor.matmul(qkps, wqk_sb[0][:, ot * 128:(ot + 1) * 128],
                                         xa, start=True, stop=False)
                        nc.tensor.matmul(qkps, wqk_sb[1][:, ot * 128:(ot + 1) * 128],
                                         xb, start=False, stop=True)
                        qk_t = qkp.tile([128, STRIP], dt.bfloat16, tag="qk_t", bufs=12)
                        nc.any.tensor_copy(qk_t, qkps)
                        qk_sb[ot] = qk_t
                        if prev is not None:
                            if i == 1:
                                tail_av(prev, 0)
                            elif i == 2:
                                tail_av(prev, 1)
                            elif i == 3:
                                tail_tp(prev, 0)

                    # ---- dots -> exp; mask-mult per (hg, half)
                    edm_sb = {}
                    ed_sb = {}
                    for j, (hg, half) in enumerate(
                            ((0, 0), (0, 1), (1, 0), (1, 1))):
                            dps = psp.tile([128, 2, 512], dt.float32,
                                           tag="dps", bufs=DPS_BUFS)
                            for wp in range(NWP):
                                c0 = wp * 128
                                for r2 in range(2):
                                    rg = 2 * half + r2
                                    nc.tensor.matmul(
                                        dps[:, r2, c0:c0 + 128],
                                        qk_sb[2 + hg][32 * rg:32 * rg + 32,
                                                      c0:c0 + 128],
                                        qk_sb[hg][32 * rg:32 * rg + 32,
                                                  c0:c0 + 128],
                                        start=True, stop=True,
                                        tile_position=(32 * rg, 0),
                                    )
                            ed = ep.tile([128, 2, 512], dt.bfloat16, tag="ed", bufs=10)
                            nc.scalar.activation(ed, dps, EXP)
                            ed_sb[(hg, half)] = ed
                            if prev is not None:
                                if j == 0:
                                    tail_tp(prev, 1)
                                elif j == 1:
                                    tail_op(prev, 0)
                                    tail_av(prev, 2)
                                elif j == 2:
                                    tail_tp(prev, 2)
                                    tail_op(prev, 1)
                                elif j == 3:
                                    tail_av(prev, 3)
                    for wh in range(2):
                        cs = slice(wh * 256, wh * 256 + 256)
                        for hg in range(2):
                            for half in range(2):
                                edm = ep.tile([128, 2, 256], dt.bfloat16,
                                              tag="edm", bufs=24)
                                gi = 2 * hg + half
                                if gi >= 4 - EDM_DVE:
                                    nc.vector.tensor_mul(
                                        edm, ed_sb[(hg, half)][:, :, cs],
                                        eb_sb[:, 2 * half:2 * half + 2, cs])
                                else:
                                    nc.gpsimd.tensor_mul(
                                        edm, ed_sb[(hg, half)][:, :, cs],
                                        eb_sb[:, 2 * half:2 * half + 2, cs])
                                edm_sb[(hg, half, wh)] = edm
                    if prev is not None:
                        tail_op(prev, 2)
                        tail_tp(prev, 3)

                    # ---- v projection: out [t=128, 8, 32] + ones col
                    v_sb = []
                    for tb in range(NWP):
                        vps = psp.tile([128, NHEADS, CH], dt.float32,
                                       tag="qkps", bufs=QKPS_BUFS)
                        nc.tensor.matmul(vps, xa[:, tb * 128:(tb + 1) * 128],
                                         wv_sb[0], start=True, stop=False)
                        nc.tensor.matmul(vps, xb[:, tb * 128:(tb + 1) * 128],
                                         wv_sb[1], start=False, stop=True)
                        v3 = vp.tile([128, NHEADS, CH + 1], dt.bfloat16,
                                     tag="v3", bufs=V3_BUFS)
                        nc.any.tensor_copy(v3[:, :, 0:CH], vps)
                        v_sb.append(v3)
                        if prev is not None and tb == 1:
                            tail_op(prev, 3)

                    fo = fop.tile([128, NWP, 256], dt.bfloat16, tag="fo",
                                  bufs=3)
                    pending.append((edm_sb, v_sb, fo, img, s, [None] * NWP,
                                    [None] * NWP))
            # drain remaining strips' tails
            for st in pending:
                for wp in range(NWP):
                    tail_av(st, wp)
                    tail_tp(st, wp)
                    tail_op(st, wp)
    nc.compile()
    return nc


def _host_prep(x, w_qkv, w_out, b_out, pos_embedding):
    ws = WS
    scale = CH ** -0.5
    xs = x.reshape(B * L, H // ws, ws, W // ws, ws, C)
    xs = xs.transpose(0, 1, 3, 2, 4, 5).reshape(IMG, T_IMG, C)
    xT = np.ascontiguousarray(xs.transpose(0, 2, 1)).astype(BF16)

    wq = (w_qkv[:, 0:256] * scale).astype(BF16)
    wk = w_qkv[:, 256:512].astype(BF16)
    wqk = np.concatenate([wq, wk], axis=1)
    wv = w_qkv[:, 512:768].astype(BF16)

    ri = _relative_indices(ws)
    bias = pos_embedding[ri[:, :, 0], ri[:, :, 1]]  # [i, j]
    ebT = np.exp(bias.astype(np.float64)).T.astype(np.float32)  # [j, i]
    ebsuper = np.zeros((128, 128), np.float32)
    ebsuper[0:64, 0:64] = ebT
    ebsuper[64:128, 64:128] = ebT
    ebrep = np.tile(ebsuper, (1, 16)).astype(BF16)

    ident = np.eye(128, dtype=BF16)

    return {
        "xT": xT,
        "wqk": np.ascontiguousarray(wqk),
        "wv": np.ascontiguousarray(wv),
        "wout": w_out.astype(BF16),
        "ebrep": ebrep,
        "ident": ident,
    }


def kernel(x, w_qkv, w_out, b_out, pos_embedding, window_size, **extra):
    from concourse.bass_utils import run_bass_kernel_spmd

    x = np.asarray(x, dtype=np.float32)
    w_qkv = np.asarray(w_qkv, dtype=np.float32)
    w_out = np.asarray(w_out, dtype=np.float32)
    b_out = np.asarray(b_out, dtype=np.float32)
    pos_embedding = np.asarray(pos_embedding, dtype=np.float32)

    prep = _host_prep(x, w_qkv, w_out, b_out, pos_embedding)

    if "nc" not in _CACHE:
        _CACHE["nc"] = _build_kernel()
    nc = _CACHE["nc"]

    in_maps = []
    for core in range(N_CORES):
        m = dict(prep)
        m["xT"] = np.ascontiguousarray(
            prep["xT"][core * IMG_PER_CORE:(core + 1) * IMG_PER_CORE])
        in_maps.append(m)

    res = run_bass_kernel_spmd(nc, in_maps, core_ids=list(range(N_CORES)))
    outs = [res.results[c]["out"] for c in range(N_CORES)]
    o = np.concatenate(outs, axis=0)  # [16, 4096, 256]
    o = o.reshape(B * L, H // WS, W // WS, WS, WS, C)
    o = o.transpose(0, 1, 3, 2, 4, 5).reshape(B, L, H, W, C)
    o = o.astype(np.float32)
    o += b_out.astype(np.float32)
    return np.ascontiguousarray(o)



# revision 34
# speedup vs baseline: 1.0994x; 1.0078x over previous
"""Trainium2 Bass kernel for BaseWindowAttention.

Problem (hardcoded): x [2,8,64,64,256] f32, w_qkv [256,768], w_out [256,256],
b_out [256], pos_embedding [15,15], window_size 8, heads 8, dim_head 32.

Strategy:
- Data parallel: 16 (b,l) images over 8 cores -> 2 images/core.
- Host: window-major channel-first bf16 transpose of x; fold softmax scale
  into w_q; precompute exp(bias) 2-window super-tile (off-diagonal zeros kill
  cross-window attention terms); b_out added host-side after the gather.
- Device per core, per strip of 512 tokens (8 windows):
  q/k projection ([o,t] layout), v projection ([t,o] layout, head-strided with
  an appended ones column for the softmax denominator), window-pair dots as
  4 row-group-packed [32,128]x[32,128] matmuls, ACT exp, GpSimd/DVE multiply
  by exp(bias) mask tile, AV matmul (fused denominator), reciprocal +
  broadcast normalize, PE transpose to [hc,t], out-projection, per-wp
  output DMA.
- PSUM banks: qk/v projections share a 2-buf [128,512] tag, dots use two
  2-bank tiles, av/transpose/out-proj share a 2-buf small tag (8 banks
  total).  Split x DMAs ([128,256] quarters) shorten prefetch latency.
  Copies routed via nc.any so the Tile scheduler balances ACT/DVE; the
  exp(bias) mask multiply is split 3 GpSimd / 1 DVE.

Perf notes (measured on trn2 via NTFF): baseline 294us -> this version
~197us.  PE (tensor engine) is the critical resource: LDWEIGHTS fully
overlaps MATMUL, so instruction *time* not load count dominates; HAM
p-state (1.2 vs 2.4 GHz) oscillates with PE idle gaps, so PSUM slot
availability (qkps bufs=2) and deep SBUF pools matter more than per-op
engine choice.  The largest single win after PSUM rebalancing was
splitting each exp(bias) mask-multiply into two wp-pair halves with
separate edm tiles, h0 halves emitted before h1 halves (218 -> 197us):
AV for wp 0-1 then gates on only half the serialized GpSimd mul chain.
EDM_DVE=1 beats 2 or 4 in both structures.  Attempted and rejected:
DMA-XBAR transpose (Sync engine serializes ~1.2us/transpose -> 2x
slower), per-window garbage-free dots via column tile_position packing
(two stationary tiles sharing a PE row band corrupt results: one
window's outputs zero out), fp8 projections (error budget too tight).
"""

import os
import sys
import numpy as np

sys.path.insert(0, "/opt/trn_rl_repo")
os.environ.setdefault("JAX_PLATFORMS", "")

import ml_dtypes

BF16 = ml_dtypes.bfloat16

B, L, H, W, C = 2, 8, 64, 64, 256
WS = 8
NHEADS = 8
CH = 32
N_CORES = 8
IMG = B * L                 # 16 images
IMG_PER_CORE = IMG // N_CORES
T_IMG = H * W               # 4096 tokens per image
STRIP = 512                 # tokens per strip (8 windows)
N_STRIPS = T_IMG // STRIP   # 8
NWP = STRIP // 128          # 4 window pairs per strip

_CACHE = {}


def _relative_indices(ws):
    idx = np.array([[i, j] for i in range(ws) for j in range(ws)])
    rel = idx[None, :, :] - idx[:, None, :] + ws - 1
    return rel


def _build_kernel():
    import concourse.bass as bass
    import concourse.mybir as mybir
    import concourse.tile as tile
    from concourse import bacc

    dt = mybir.dt
    nc = bacc.Bacc("TRN2", target_bir_lowering=False, debug=False)

    xT = nc.dram_tensor("xT", [IMG_PER_CORE, C, T_IMG], dt.bfloat16,
                        kind="ExternalInput").ap()
    wqk = nc.dram_tensor("wqk", [C, 512], dt.bfloat16, kind="ExternalInput").ap()
    wv = nc.dram_tensor("wv", [C, C], dt.bfloat16, kind="ExternalInput").ap()
    wout = nc.dram_tensor("wout", [C, C], dt.bfloat16, kind="ExternalInput").ap()
    ebrep = nc.dram_tensor("ebrep", [128, 2048], dt.bfloat16,
                           kind="ExternalInput").ap()
    ident = nc.dram_tensor("ident", [128, 128], dt.bfloat16,
                           kind="ExternalInput").ap()
    out = nc.dram_tensor("out", [IMG_PER_CORE, T_IMG, C], dt.bfloat16,
                         kind="ExternalOutput").ap()

    EXP = mybir.ActivationFunctionType.Exp

    with tile.TileContext(nc) as tc:
        from contextlib import ExitStack
        with ExitStack() as ctx:
            consts = ctx.enter_context(tc.tile_pool(name="consts", bufs=1))
            xp = ctx.enter_context(tc.tile_pool(name="xp", bufs=3))
            qkp = ctx.enter_context(tc.tile_pool(name="qkp", bufs=8))
            vp = ctx.enter_context(tc.tile_pool(name="vp", bufs=8))
            ep = ctx.enter_context(tc.tile_pool(name="ep", bufs=3))
            anp = ctx.enter_context(tc.tile_pool(name="anp", bufs=3))
            aotp = ctx.enter_context(tc.tile_pool(name="aotp", bufs=4))
            rdp = ctx.enter_context(tc.tile_pool(name="rdp", bufs=3))
            fop = ctx.enter_context(tc.tile_pool(name="fop", bufs=3))
            psp = ctx.enter_context(tc.tile_pool(name="psp", bufs=1, space="PSUM"))

            # ---- strip-0 inputs first (head latency), then constants.
            # Big constants are split so no single DMA engine serializes
            # a large transfer in front of the first matmuls.
            pre_x = []
            for half in range(2):
                xt = xp.tile([128, STRIP], dt.bfloat16,
                             tag=("xa" if half == 0 else "xb"), bufs=6)
                for q in range(2):
                    nc.sync.dma_start(
                        out=xt[:, q * 256:(q + 1) * 256],
                        in_=xT[0, half * 128:half * 128 + 128,
                               q * 256:(q + 1) * 256])
                pre_x.append(xt)

            # wqk first (gates the very first matmuls), then wv, then the
            # rest of the constants
            wqk_sb = []
            wv_sb = []
            wout_sb = []
            for kk in range(2):
                wqk_t = consts.tile([128, 512], dt.bfloat16, tag=f"wqk{kk}")
                for q in range(2):
                    nc.sync.dma_start(
                        out=wqk_t[:, q * 256:(q + 1) * 256],
                        in_=wqk[kk * 128:(kk + 1) * 128,
                                q * 256:(q + 1) * 256])
                wqk_sb.append(wqk_t)
            for kk in range(2):
                wv_t = consts.tile([128, 256], dt.bfloat16, tag=f"wv{kk}")
                nc.sync.dma_start(out=wv_t, in_=wv[kk * 128:(kk + 1) * 128, :])
                wv_sb.append(wv_t)
            for kk in range(2):
                wout_t = consts.tile([128, 256], dt.bfloat16, tag=f"wout{kk}")
                nc.sync.dma_start(out=wout_t, in_=wout[kk * 128:(kk + 1) * 128, :])
                wout_sb.append(wout_t)
            eb_sb = consts.tile([128, 4, 512], dt.bfloat16, tag="eb")
            ebr = ebrep.rearrange("p (r c) -> p r c", r=4)
            for r in range(4):
                nc.sync.dma_start(out=eb_sb[:, r, :], in_=ebr[:, r, :])
            id_sb = consts.tile([128, 128], dt.bfloat16, tag="id")
            nc.sync.dma_start(out=id_sb, in_=ident)

            QKPS_BUFS = int(os.environ.get("QKPS_BUFS", "2"))
            SMALLPS_BUFS = int(os.environ.get("SMALLPS_BUFS", "2"))
            DPS_BUFS = int(os.environ.get("DPS_BUFS", "2"))
            EDM_DVE = int(os.environ.get("EDM_DVE", "1"))  # groups on DVE (of 4)
            V3_BUFS = 12
            # ones columns of the v3 ring are written once here; the loop
            # only ever writes [:, :, 0:CH], so they persist across reuse
            for _ in range(V3_BUFS):
                v3i = vp.tile([128, NHEADS, CH + 1], dt.bfloat16,
                              tag="v3", bufs=V3_BUFS)
                nc.gpsimd.memset(v3i[:, :, CH:CH + 1], 1.0)

            orr = out.rearrange("i (s w p) c -> i s p w c",
                                s=N_STRIPS, w=NWP)

            # ---- software pipeline: the per-wp tail chain of strip s-1
            # (AV -> normalize -> transpose -> out-proj -> store) ping-pongs
            # PE<->DVE with no slack, and the in-order PE queue head-of-line
            # blocks on each hop.  Split each tail into three sub-phases and
            # interleave them between the front-phase matmul groups of strip
            # s, so every dependent PE op reaches the queue head with its
            # cross-engine producer long finished.
            def tail_av(st, wp):
                """AV matmuls (+denominator col) and normalize for one wp."""
                edm_sb, v_sb, fo, img_, s_, attn_sb, aot_sbs = st
                cc = (wp % 2) * 128
                avps = psp.tile([128, NHEADS, CH + 1], dt.float32,
                                tag="smallps", bufs=SMALLPS_BUFS)
                for hg in range(2):
                    for rg in range(4):
                        h = 4 * hg + rg
                        nc.tensor.matmul(
                            avps[:, h, :],
                            edm_sb[(hg, rg // 2, wp // 2)][:, rg % 2,
                                                           cc:cc + 128],
                            v_sb[wp][:, h, :],
                            start=True, stop=True,
                        )
                rd = rdp.tile([128, NHEADS, 1], dt.float32, tag="rd", bufs=8)
                nc.vector.reciprocal(rd, avps[:, :, CH:CH + 1])
                attn = anp.tile([128, NHEADS, CH], dt.bfloat16,
                                tag="attn", bufs=8)
                nc.vector.tensor_mul(attn, avps[:, :, 0:CH],
                                     rd.to_broadcast((128, NHEADS, CH)))
                attn_sb[wp] = attn

            def tail_tp(st, wp):
                """transpose [t,hc] -> [hc,t] + evacuate for one wp."""
                edm_sb, v_sb, fo, img_, s_, attn_sb, aot_sbs = st
                attn = attn_sb[wp]
                tps = psp.tile([128, 2, 128], dt.bfloat16,
                               tag="smallps", bufs=SMALLPS_BUFS)
                for half in range(2):
                    nc.tensor.transpose(
                        tps[:, half, :],
                        attn[:, half * 4:(half + 1) * 4, :], id_sb)
                aot = aotp.tile([128, 2, 128], dt.bfloat16, tag="aot", bufs=8)
                nc.any.tensor_copy(aot, tps)
                aot_sbs[wp] = aot

            def tail_op(st, wp):
                """out-projection + store copy for one wp (store DMA on
                the last wp)."""
                edm_sb, v_sb, fo, img_, s_, attn_sb, aot_sbs = st
                aot = aot_sbs[wp]
                ops = psp.tile([128, 256], dt.float32, tag="smallps",
                               bufs=SMALLPS_BUFS)
                nc.tensor.matmul(ops, aot[:, 0, :], wout_sb[0],
                                 start=True, stop=False)
                nc.tensor.matmul(ops, aot[:, 1, :], wout_sb[1],
                                 start=False, stop=True)
                nc.any.tensor_copy(fo[:, wp, :], ops)
                if wp == NWP - 1:
                    nc.sync.dma_start(out=orr[img_, s_], in_=fo)

            PIPE_LAG = int(os.environ.get("PIPE_LAG", "1"))
            pending = []   # oldest-first states awaiting tails
            prev = None
            strips = [(img, s) for img in range(IMG_PER_CORE)
                      for s in range(N_STRIPS)]
            for img, s in strips:
                    prev = pending.pop(0) if len(pending) >= PIPE_LAG else None
                    t0 = s * STRIP
                    if img == 0 and s == 0:
                        xa, xb = pre_x
                    else:
                        xa = xp.tile([128, STRIP], dt.bfloat16, tag="xa", bufs=6)
                        nc.sync.dma_start(out=xa,
                                          in_=xT[img, 0:128, t0:t0 + 512])
                        xb = xp.tile([128, STRIP], dt.bfloat16, tag="xb", bufs=6)
                        nc.sync.dma_start(out=xb,
                                          in_=xT[img, 128:256, t0:t0 + 512])

                    # ---- q/k projection: out [o=128 (4 heads), t=512]
                    # order q03, k03, q47, k47: head-group 0's dots need
                    # only the first two copies
                    qk_sb = [None] * 4
                    for i, ot in enumerate((0, 2, 1, 3)):
                        qkps = psp.tile([128, STRIP], dt.float32, tag="qkps",
                                        bufs=QKPS_BUFS)
                        nc.tensor.matmul(qkps, wqk_sb[0][:, ot * 128:(ot + 1) * 128],
                                         xa, start=True, stop=False)
                        nc.tensor.matmul(qkps, wqk_sb[1][:, ot * 128:(ot + 1) * 128],
                                         xb, start=False, stop=True)
                        qk_t = qkp.tile([128, STRIP], dt.bfloat16, tag="qk_t", bufs=12)
                        nc.any.tensor_copy(qk_t, qkps)
                        qk_sb[ot] = qk_t
                        if prev is not None:
                            if i == 1:
                                tail_av(prev, 0)
                            elif i == 2:
                                tail_av(prev, 1)
                            elif i == 3:
                                tail_tp(prev, 0)

                    # ---- dots -> exp; mask-mult per (hg, half)
                    edm_sb = {}
                    ed_sb = {}
                    for j, (hg, half) in enumerate(
                            ((0, 0), (0, 1), (1, 0), (1, 1))):
                            dps = psp.tile([128, 2, 512], dt.float32,
                                           tag="dps", bufs=DPS_BUFS)
                            for wp in range(NWP):
                                c0 = wp * 128
                                for r2 in range(2):
                                    rg = 2 * half + r2
                                    nc.tensor.matmul(
                                        dps[:, r2, c0:c0 + 128],
                                        qk_sb[2 + hg][32 * rg:32 * rg + 32,
                                                      c0:c0 + 128],
                                        qk_sb[hg][32 * rg:32 * rg + 32,
                                                  c0:c0 + 128],
                                        start=True, stop=True,
                                        tile_position=(32 * rg, 0),
                                    )
                            ed = ep.tile([128, 2, 512], dt.bfloat16, tag="ed", bufs=10)
                            nc.scalar.activation(ed, dps, EXP)
                            ed_sb[(hg, half)] = ed
                            if prev is not None:
                                if j == 0:
                                    tail_tp(prev, 1)
                                elif j == 1:
                                    tail_op(prev, 0)
                                    tail_av(prev, 2)
                                elif j == 2:
                                    tail_tp(prev, 2)
                                    tail_op(prev, 1)
                                elif j == 3:
                                    tail_av(prev, 3)
                    for wh in range(2):
                        cs = slice(wh * 256, wh * 256 + 256)
                        for hg in range(2):
                            for half in range(2):
                                edm = ep.tile([128, 2, 256], dt.bfloat16,
                                              tag="edm", bufs=24)
                                gi = 2 * hg + half
                                if gi >= 4 - EDM_DVE:
                                    nc.vector.tensor_mul(
                                        edm, ed_sb[(hg, half)][:, :, cs],
                                        eb_sb[:, 2 * half:2 * half + 2, cs])
                                else:
                                    nc.gpsimd.tensor_mul(
                                        edm, ed_sb[(hg, half)][:, :, cs],
                                        eb_sb[:, 2 * half:2 * half + 2, cs])
                                edm_sb[(hg, half, wh)] = edm
                    if prev is not None:
                        tail_op(prev, 2)
                        tail_tp(prev, 3)

                    # ---- v projection: out [t=128, 8, 32] + ones col
                    v_sb = []
                    for tb in range(NWP):
                        vps = psp.tile([128, NHEADS, CH], dt.float32,
                                       tag="qkps", bufs=QKPS_BUFS)
                        nc.tensor.matmul(vps, xa[:, tb * 128:(tb + 1) * 128],
                                         wv_sb[0], start=True, stop=False)
                        nc.tensor.matmul(vps, xb[:, tb * 128:(tb + 1) * 128],
                                         wv_sb[1], start=False, stop=True)
                        v3 = vp.tile([128, NHEADS, CH + 1], dt.bfloat16,
                                     tag="v3", bufs=V3_BUFS)
                        nc.any.tensor_copy(v3[:, :, 0:CH], vps)
                        v_sb.append(v3)
                        if prev is not None and tb == 1:
                            tail_op(prev, 3)

                    fo = fop.tile([128, NWP, 256], dt.bfloat16, tag="fo",
                                  bufs=3)
                    pending.append((edm_sb, v_sb, fo, img, s, [None] * NWP,
                                    [None] * NWP))
            # drain remaining strips' tails, phase-grouped so each
            # cross-engine dependency has ~3 window-pairs of slack
            for st in pending:
                for wp in range(NWP):
                    tail_av(st, wp)
                for wp in range(NWP):
                    tail_tp(st, wp)
                for wp in range(NWP):
                    tail_op(st, wp)
    nc.compile()
    return nc


def _host_prep(x, w_qkv, w_out, b_out, pos_embedding):
    ws = WS
    scale = CH ** -0.5
    xs = x.reshape(B * L, H // ws, ws, W // ws, ws, C)
    xs = xs.transpose(0, 1, 3, 2, 4, 5).reshape(IMG, T_IMG, C)
    xT = np.ascontiguousarray(xs.transpose(0, 2, 1)).astype(BF16)

    wq = (w_qkv[:, 0:256] * scale).astype(BF16)
    wk = w_qkv[:, 256:512].astype(BF16)
    wqk = np.concatenate([wq, wk], axis=1)
    wv = w_qkv[:, 512:768].astype(BF16)

    ri = _relative_indices(ws)
    bias = pos_embedding[ri[:, :, 0], ri[:, :, 1]]  # [i, j]
    ebT = np.exp(bias.astype(np.float64)).T.astype(np.float32)  # [j, i]
    ebsuper = np.zeros((128, 128), np.float32)
    ebsuper[0:64, 0:64] = ebT
    ebsuper[64:128, 64:128] = ebT
    ebrep = np.tile(ebsuper, (1, 16)).astype(BF16)

    ident = np.eye(128, dtype=BF16)

    return {
        "xT": xT,
        "wqk": np.ascontiguousarray(wqk),
        "wv": np.ascontiguousarray(wv),
        "wout": w_out.astype(BF16),
        "ebrep": ebrep,
        "ident": ident,
    }


def kernel(x, w_qkv, w_out, b_out, pos_embedding, window_size, **extra):
    from concourse.bass_utils import run_bass_kernel_spmd

    x = np.asarray(x, dtype=np.float32)
    w_qkv = np.asarray(w_qkv, dtype=np.float32)
    w_out = np.asarray(w_out, dtype=np.float32)
    b_out = np.asarray(b_out, dtype=np.float32)
    pos_embedding = np.asarray(pos_embedding, dtype=np.float32)

    prep = _host_prep(x, w_qkv, w_out, b_out, pos_embedding)

    if "nc" not in _CACHE:
        _CACHE["nc"] = _build_kernel()
    nc = _CACHE["nc"]

    in_maps = []
    for core in range(N_CORES):
        m = dict(prep)
        m["xT"] = np.ascontiguousarray(
            prep["xT"][core * IMG_PER_CORE:(core + 1) * IMG_PER_CORE])
        in_maps.append(m)

    res = run_bass_kernel_spmd(nc, in_maps, core_ids=list(range(N_CORES)))
    outs = [res.results[c]["out"] for c in range(N_CORES)]
    o = np.concatenate(outs, axis=0)  # [16, 4096, 256]
    o = o.reshape(B * L, H // WS, W // WS, WS, WS, C)
    o = o.transpose(0, 1, 3, 2, 4, 5).reshape(B, L, H, W, C)
    o = o.astype(np.float32)
    o += b_out.astype(np.float32)
    return np.ascontiguousarray(o)



# revision 35
# speedup vs baseline: 1.1153x; 1.0145x over previous
"""Trainium2 Bass kernel for BaseWindowAttention.

Problem (hardcoded): x [2,8,64,64,256] f32, w_qkv [256,768], w_out [256,256],
b_out [256], pos_embedding [15,15], window_size 8, heads 8, dim_head 32.

Strategy:
- Data parallel: 16 (b,l) images over 8 cores -> 2 images/core.
- Host: window-major channel-first bf16 transpose of x; fold softmax scale
  into w_q; precompute exp(bias) 2-window super-tile (off-diagonal zeros kill
  cross-window attention terms); b_out added host-side after the gather.
- Device per core, per strip of 512 tokens (8 windows):
  q/k projection ([o,t] layout), v projection ([t,o] layout, head-strided with
  an appended ones column for the softmax denominator), window-pair dots as
  4 row-group-packed [32,128]x[32,128] matmuls, ACT exp, GpSimd/DVE multiply
  by exp(bias) mask tile, AV matmul (fused denominator), reciprocal +
  broadcast normalize, PE transpose to [hc,t], out-projection, per-wp
  output DMA.
- PSUM banks: qk/v projections share a 2-buf [128,512] tag, dots use two
  2-bank tiles, av/transpose/out-proj share a 2-buf small tag (8 banks
  total).  Split x DMAs ([128,256] quarters) shorten prefetch latency.
  Copies routed via nc.any so the Tile scheduler balances ACT/DVE; the
  exp(bias) mask multiply is split 3 GpSimd / 1 DVE.

Perf notes (measured on trn2 via NTFF; see hwtime.py for the ctypes
axon_start_nrt_profile harness): 294us -> 197us (prior session) ->
~164us (this session).  The big win this session was SOFTWARE
PIPELINING the strips: every engine queue is strict in-order, so the
per-wp tail chain (AV -> recip/normalize -> PE transpose -> out-proj
-> store) used to head-of-line-block PE for ~600-1200ns at each
PE<->DVE hop (~67us of PE idle + HAM re-throttle).  Each tail is now
split into three sub-phases (tail_av / tail_tp / tail_op) and
interleaved at fixed slots between the matmul groups of the NEXT
strip's front phase, so every dependent PE op reaches the queue head
with its cross-engine producer long finished (197 -> 167us).  Smaller
wins: batched per-strip output DMA + merged x loads + hoisted v3
ones-column memsets (HWDGE descriptor gen is 625ns each, serialized),
wqk-first constant DMA order, phase-grouped drain (-> ~164us).
Measured on HW: every LDWEIGHTS costs ~95-105ns regardless of row
count (no FWL on this toolchain; 1536 LDWs = 150us serialized is the
PE floor), MATMUL issue-to-issue for a stream is ~N/2.4 + ~30ns.
Attempted and rejected THIS session: EDM_DVE=0/2 (176us vs 167),
PIPE_LAG=2 (172), merged full-width mask muls (178), v-proj before
mask (179), transposes as regular matmul-by-identity + f32 tps (sim
likes it, HW ~+6us: the f32 aot CAST copies load DVE, the tighter
engine), bigger edm/v3/fo buf rings (180?! deeper rings hurt),
DPS_BUFS=1 or SMALLPS_BUFS=3/4 (psum is bank-granular, 8 banks full).
Prior session's rejects still stand: DMA-XBAR transpose, per-window
garbage-free dots via column tile_position packing (corrupts; also
would double LDW count which is the HW floor), fp8 (error budget).
"""

import os
import sys
import numpy as np

sys.path.insert(0, "/opt/trn_rl_repo")
os.environ.setdefault("JAX_PLATFORMS", "")

import ml_dtypes

BF16 = ml_dtypes.bfloat16

B, L, H, W, C = 2, 8, 64, 64, 256
WS = 8
NHEADS = 8
CH = 32
N_CORES = 8
IMG = B * L                 # 16 images
IMG_PER_CORE = IMG // N_CORES
T_IMG = H * W               # 4096 tokens per image
STRIP = 512                 # tokens per strip (8 windows)
N_STRIPS = T_IMG // STRIP   # 8
NWP = STRIP // 128          # 4 window pairs per strip

_CACHE = {}


def _relative_indices(ws):
    idx = np.array([[i, j] for i in range(ws) for j in range(ws)])
    rel = idx[None, :, :] - idx[:, None, :] + ws - 1
    return rel


def _build_kernel():
    import concourse.bass as bass
    import concourse.mybir as mybir
    import concourse.tile as tile
    from concourse import bacc

    dt = mybir.dt
    nc = bacc.Bacc("TRN2", target_bir_lowering=False, debug=False)

    xT = nc.dram_tensor("xT", [IMG_PER_CORE, C, T_IMG], dt.bfloat16,
                        kind="ExternalInput").ap()
    wqk = nc.dram_tensor("wqk", [C, 512], dt.bfloat16, kind="ExternalInput").ap()
    wv = nc.dram_tensor("wv", [C, C], dt.bfloat16, kind="ExternalInput").ap()
    wout = nc.dram_tensor("wout", [C, C], dt.bfloat16, kind="ExternalInput").ap()
    ebrep = nc.dram_tensor("ebrep", [128, 2048], dt.bfloat16,
                           kind="ExternalInput").ap()
    ident = nc.dram_tensor("ident", [128, 128], dt.bfloat16,
                           kind="ExternalInput").ap()
    out = nc.dram_tensor("out", [IMG_PER_CORE, T_IMG, C], dt.bfloat16,
                         kind="ExternalOutput").ap()

    EXP = mybir.ActivationFunctionType.Exp

    with tile.TileContext(nc) as tc:
        from contextlib import ExitStack
        with ExitStack() as ctx:
            consts = ctx.enter_context(tc.tile_pool(name="consts", bufs=1))
            xp = ctx.enter_context(tc.tile_pool(name="xp", bufs=3))
            qkp = ctx.enter_context(tc.tile_pool(name="qkp", bufs=8))
            vp = ctx.enter_context(tc.tile_pool(name="vp", bufs=8))
            ep = ctx.enter_context(tc.tile_pool(name="ep", bufs=3))
            anp = ctx.enter_context(tc.tile_pool(name="anp", bufs=3))
            aotp = ctx.enter_context(tc.tile_pool(name="aotp", bufs=4))
            rdp = ctx.enter_context(tc.tile_pool(name="rdp", bufs=3))
            fop = ctx.enter_context(tc.tile_pool(name="fop", bufs=3))
            psp = ctx.enter_context(tc.tile_pool(name="psp", bufs=1, space="PSUM"))

            # ---- strip-0 inputs first (head latency), then constants.
            # Big constants are split so no single DMA engine serializes
            # a large transfer in front of the first matmuls.
            pre_x = []
            for half in range(2):
                xt = xp.tile([128, STRIP], dt.bfloat16,
                             tag=("xa" if half == 0 else "xb"), bufs=6)
                for q in range(2):
                    nc.sync.dma_start(
                        out=xt[:, q * 256:(q + 1) * 256],
                        in_=xT[0, half * 128:half * 128 + 128,
                               q * 256:(q + 1) * 256])
                pre_x.append(xt)

            # wqk first (gates the very first matmuls), then wv, then the
            # rest of the constants
            wqk_sb = []
            wv_sb = []
            wout_sb = []
            for kk in range(2):
                wqk_t = consts.tile([128, 512], dt.bfloat16, tag=f"wqk{kk}")
                for q in range(2):
                    nc.sync.dma_start(
                        out=wqk_t[:, q * 256:(q + 1) * 256],
                        in_=wqk[kk * 128:(kk + 1) * 128,
                                q * 256:(q + 1) * 256])
                wqk_sb.append(wqk_t)
            for kk in range(2):
                wv_t = consts.tile([128, 256], dt.bfloat16, tag=f"wv{kk}")
                nc.sync.dma_start(out=wv_t, in_=wv[kk * 128:(kk + 1) * 128, :])
                wv_sb.append(wv_t)
            for kk in range(2):
                wout_t = consts.tile([128, 256], dt.bfloat16, tag=f"wout{kk}")
                nc.sync.dma_start(out=wout_t, in_=wout[kk * 128:(kk + 1) * 128, :])
                wout_sb.append(wout_t)
            eb_sb = consts.tile([128, 4, 512], dt.bfloat16, tag="eb")
            ebr = ebrep.rearrange("p (r c) -> p r c", r=4)
            for r in range(4):
                nc.sync.dma_start(out=eb_sb[:, r, :], in_=ebr[:, r, :])
            id_sb = consts.tile([128, 128], dt.bfloat16, tag="id")
            nc.sync.dma_start(out=id_sb, in_=ident)

            QKPS_BUFS = int(os.environ.get("QKPS_BUFS", "2"))
            SMALLPS_BUFS = int(os.environ.get("SMALLPS_BUFS", "2"))
            DPS_BUFS = int(os.environ.get("DPS_BUFS", "2"))
            EDM_DVE = int(os.environ.get("EDM_DVE", "1"))  # groups on DVE (of 4)
            V3_BUFS = 12
            # ones columns of the v3 ring are written once here; the loop
            # only ever writes [:, :, 0:CH], so they persist across reuse
            for _ in range(V3_BUFS):
                v3i = vp.tile([128, NHEADS, CH + 1], dt.bfloat16,
                              tag="v3", bufs=V3_BUFS)
                nc.gpsimd.memset(v3i[:, :, CH:CH + 1], 1.0)

            orr = out.rearrange("i (s w p) c -> i s p w c",
                                s=N_STRIPS, w=NWP)

            # ---- software pipeline: the per-wp tail chain of strip s-1
            # (AV -> normalize -> transpose -> out-proj -> store) ping-pongs
            # PE<->DVE with no slack, and the in-order PE queue head-of-line
            # blocks on each hop.  Split each tail into three sub-phases and
            # interleave them between the front-phase matmul groups of strip
            # s, so every dependent PE op reaches the queue head with its
            # cross-engine producer long finished.
            def tail_av(st, wp):
                """AV matmuls (+denominator col) and normalize for one wp."""
                edm_sb, v_sb, fo, img_, s_, attn_sb, aot_sbs = st
                cc = (wp % 2) * 128
                avps = psp.tile([128, NHEADS, CH + 1], dt.float32,
                                tag="smallps", bufs=SMALLPS_BUFS)
                for hg in range(2):
                    for rg in range(4):
                        h = 4 * hg + rg
                        nc.tensor.matmul(
                            avps[:, h, :],
                            edm_sb[(hg, rg // 2, wp // 2)][:, rg % 2,
                                                           cc:cc + 128],
                            v_sb[wp][:, h, :],
                            start=True, stop=True,
                        )
                rd = rdp.tile([128, NHEADS, 1], dt.float32, tag="rd", bufs=8)
                nc.vector.reciprocal(rd, avps[:, :, CH:CH + 1])
                attn = anp.tile([128, NHEADS, CH], dt.bfloat16,
                                tag="attn", bufs=8)
                nc.vector.tensor_mul(attn, avps[:, :, 0:CH],
                                     rd.to_broadcast((128, NHEADS, CH)))
                attn_sb[wp] = attn

            def tail_tp(st, wp):
                """transpose [t,hc] -> [hc,t] + evacuate for one wp."""
                edm_sb, v_sb, fo, img_, s_, attn_sb, aot_sbs = st
                attn = attn_sb[wp]
                tps = psp.tile([128, 2, 128], dt.bfloat16,
                               tag="smallps", bufs=SMALLPS_BUFS)
                for half in range(2):
                    nc.tensor.transpose(
                        tps[:, half, :],
                        attn[:, half * 4:(half + 1) * 4, :], id_sb)
                aot = aotp.tile([128, 2, 128], dt.bfloat16, tag="aot", bufs=8)
                nc.any.tensor_copy(aot, tps)
                aot_sbs[wp] = aot

            def tail_op(st, wp):
                """out-projection + store copy for one wp (store DMA on
                the last wp)."""
                edm_sb, v_sb, fo, img_, s_, attn_sb, aot_sbs = st
                aot = aot_sbs[wp]
                ops = psp.tile([128, 256], dt.float32, tag="smallps",
                               bufs=SMALLPS_BUFS)
                nc.tensor.matmul(ops, aot[:, 0, :], wout_sb[0],
                                 start=True, stop=False)
                nc.tensor.matmul(ops, aot[:, 1, :], wout_sb[1],
                                 start=False, stop=True)
                nc.any.tensor_copy(fo[:, wp, :], ops)
                if wp == NWP - 1:
                    nc.sync.dma_start(out=orr[img_, s_], in_=fo)

            PIPE_LAG = int(os.environ.get("PIPE_LAG", "1"))
            pending = []   # oldest-first states awaiting tails
            prev = None
            strips = [(img, s) for img in range(IMG_PER_CORE)
                      for s in range(N_STRIPS)]
            for img, s in strips:
                    prev = pending.pop(0) if len(pending) >= PIPE_LAG else None
                    t0 = s * STRIP
                    if img == 0 and s == 0:
                        xa, xb = pre_x
                    else:
                        xa = xp.tile([128, STRIP], dt.bfloat16, tag="xa", bufs=6)
                        nc.sync.dma_start(out=xa,
                                          in_=xT[img, 0:128, t0:t0 + 512])
                        xb = xp.tile([128, STRIP], dt.bfloat16, tag="xb", bufs=6)
                        nc.sync.dma_start(out=xb,
                                          in_=xT[img, 128:256, t0:t0 + 512])

                    # ---- q/k projection: out [o=128 (4 heads), t=512]
                    # order q03, k03, q47, k47: head-group 0's dots need
                    # only the first two copies
                    qk_sb = [None] * 4
                    for i, ot in enumerate((0, 2, 1, 3)):
                        qkps = psp.tile([128, STRIP], dt.float32, tag="qkps",
                                        bufs=QKPS_BUFS)
                        nc.tensor.matmul(qkps, wqk_sb[0][:, ot * 128:(ot + 1) * 128],
                                         xa, start=True, stop=False)
                        nc.tensor.matmul(qkps, wqk_sb[1][:, ot * 128:(ot + 1) * 128],
                                         xb, start=False, stop=True)
                        qk_t = qkp.tile([128, STRIP], dt.bfloat16, tag="qk_t", bufs=12)
                        nc.any.tensor_copy(qk_t, qkps)
                        qk_sb[ot] = qk_t
                        if prev is not None:
                            if i == 1:
                                tail_av(prev, 0)
                            elif i == 2:
                                tail_av(prev, 1)
                            elif i == 3:
                                tail_tp(prev, 0)

                    # ---- dots -> exp; mask-mult per (hg, half)
                    edm_sb = {}
                    ed_sb = {}
                    for j, (hg, half) in enumerate(
                            ((0, 0), (0, 1), (1, 0), (1, 1))):
                            dps = psp.tile([128, 2, 512], dt.float32,
                                           tag="dps", bufs=DPS_BUFS)
                            for wp in range(NWP):
                                c0 = wp * 128
                                for r2 in range(2):
                                    rg = 2 * half + r2
                                    nc.tensor.matmul(
                                        dps[:, r2, c0:c0 + 128],
                                        qk_sb[2 + hg][32 * rg:32 * rg + 32,
                                                      c0:c0 + 128],
                                        qk_sb[hg][32 * rg:32 * rg + 32,
                                                  c0:c0 + 128],
                                        start=True, stop=True,
                                        tile_position=(32 * rg, 0),
                                    )
                            ed = ep.tile([128, 2, 512], dt.bfloat16, tag="ed", bufs=10)
                            nc.scalar.activation(ed, dps, EXP)
                            ed_sb[(hg, half)] = ed
                            if prev is not None:
                                if j == 0:
                                    tail_tp(prev, 1)
                                elif j == 1:
                                    tail_op(prev, 0)
                                    tail_av(prev, 2)
                                elif j == 2:
                                    tail_tp(prev, 2)
                                    tail_op(prev, 1)
                                elif j == 3:
                                    tail_av(prev, 3)
                    for wh in range(2):
                        cs = slice(wh * 256, wh * 256 + 256)
                        for hg in range(2):
                            for half in range(2):
                                edm = ep.tile([128, 2, 256], dt.bfloat16,
                                              tag="edm", bufs=24)
                                gi = 2 * hg + half
                                if gi >= 4 - EDM_DVE:
                                    nc.vector.tensor_mul(
                                        edm, ed_sb[(hg, half)][:, :, cs],
                                        eb_sb[:, 2 * half:2 * half + 2, cs])
                                else:
                                    nc.gpsimd.tensor_mul(
                                        edm, ed_sb[(hg, half)][:, :, cs],
                                        eb_sb[:, 2 * half:2 * half + 2, cs])
                                edm_sb[(hg, half, wh)] = edm
                    if prev is not None:
                        tail_op(prev, 2)
                        tail_tp(prev, 3)

                    # ---- v projection: out [t=128, 8, 32] + ones col
                    v_sb = []
                    for tb in range(NWP):
                        vps = psp.tile([128, NHEADS, CH], dt.float32,
                                       tag="qkps", bufs=QKPS_BUFS)
                        nc.tensor.matmul(vps, xa[:, tb * 128:(tb + 1) * 128],
                                         wv_sb[0], start=True, stop=False)
                        nc.tensor.matmul(vps, xb[:, tb * 128:(tb + 1) * 128],
                                         wv_sb[1], start=False, stop=True)
                        v3 = vp.tile([128, NHEADS, CH + 1], dt.bfloat16,
                                     tag="v3", bufs=V3_BUFS)
                        nc.any.tensor_copy(v3[:, :, 0:CH], vps)
                        v_sb.append(v3)
                        if prev is not None and tb == 1:
                            tail_op(prev, 3)

                    fo = fop.tile([128, NWP, 256], dt.bfloat16, tag="fo",
                                  bufs=3)
                    pending.append((edm_sb, v_sb, fo, img, s, [None] * NWP,
                                    [None] * NWP))
            # drain remaining strips' tails, phase-grouped so each
            # cross-engine dependency has ~3 window-pairs of slack
            for st in pending:
                for wp in range(NWP):
                    tail_av(st, wp)
                for wp in range(NWP):
                    tail_tp(st, wp)
                for wp in range(NWP):
                    tail_op(st, wp)
    nc.compile()
    return nc


def _host_prep(x, w_qkv, w_out, b_out, pos_embedding):
    ws = WS
    scale = CH ** -0.5
    xs = x.reshape(B * L, H // ws, ws, W // ws, ws, C)
    xs = xs.transpose(0, 1, 3, 2, 4, 5).reshape(IMG, T_IMG, C)
    xT = np.ascontiguousarray(xs.transpose(0, 2, 1)).astype(BF16)

    wq = (w_qkv[:, 0:256] * scale).astype(BF16)
    wk = w_qkv[:, 256:512].astype(BF16)
    wqk = np.concatenate([wq, wk], axis=1)
    wv = w_qkv[:, 512:768].astype(BF16)

    ri = _relative_indices(ws)
    bias = pos_embedding[ri[:, :, 0], ri[:, :, 1]]  # [i, j]
    ebT = np.exp(bias.astype(np.float64)).T.astype(np.float32)  # [j, i]
    ebsuper = np.zeros((128, 128), np.float32)
    ebsuper[0:64, 0:64] = ebT
    ebsuper[64:128, 64:128] = ebT
    ebrep = np.tile(ebsuper, (1, 16)).astype(BF16)

    ident = np.eye(128, dtype=BF16)

    return {
        "xT": xT,
        "wqk": np.ascontiguousarray(wqk),
        "wv": np.ascontiguousarray(wv),
        "wout": w_out.astype(BF16),
        "ebrep": ebrep,
        "ident": ident,
    }


def kernel(x, w_qkv, w_out, b_out, pos_embedding, window_size, **extra):
    from concourse.bass_utils import run_bass_kernel_spmd

    x = np.asarray(x, dtype=np.float32)
    w_qkv = np.asarray(w_qkv, dtype=np.float32)
    w_out = np.asarray(w_out, dtype=np.float32)
    b_out = np.asarray(b_out, dtype=np.float32)
    pos_embedding = np.asarray(pos_embedding, dtype=np.float32)

    prep = _host_prep(x, w_qkv, w_out, b_out, pos_embedding)

    if "nc" not in _CACHE:
        _CACHE["nc"] = _build_kernel()
    nc = _CACHE["nc"]

    in_maps = []
    for core in range(N_CORES):
        m = dict(prep)
        m["xT"] = np.ascontiguousarray(
            prep["xT"][core * IMG_PER_CORE:(core + 1) * IMG_PER_CORE])
        in_maps.append(m)

    res = run_bass_kernel_spmd(nc, in_maps, core_ids=list(range(N_CORES)))
    outs = [res.results[c]["out"] for c in range(N_CORES)]
    o = np.concatenate(outs, axis=0)  # [16, 4096, 256]
    o = o.reshape(B * L, H // WS, W // WS, WS, WS, C)
    o = o.transpose(0, 1, 3, 2, 4, 5).reshape(B, L, H, W, C)
    o = o.astype(np.float32)
    o += b_out.astype(np.float32)
    return np.ascontiguousarray(o)



# revision 38
# speedup vs baseline: 1.1178x; 1.0022x over previous
"""Trainium2 Bass kernel for BaseWindowAttention.

Problem (hardcoded): x [2,8,64,64,256] f32, w_qkv [256,768], w_out [256,256],
b_out [256], pos_embedding [15,15], window_size 8, heads 8, dim_head 32.

Strategy:
- Data parallel: 16 (b,l) images over 8 cores -> 2 images/core.
- Host: window-major channel-first bf16 transpose of x; fold softmax scale
  into w_q; precompute exp(bias) 2-window super-tile (off-diagonal zeros kill
  cross-window attention terms); b_out added host-side after the gather.
- Device per core, per strip of 512 tokens (8 windows):
  q/k projection ([o,t] layout), v projection ([t,o] layout, head-strided with
  an appended ones column for the softmax denominator), window-pair dots as
  4 row-group-packed [32,128]x[32,128] matmuls, ACT exp, GpSimd/DVE multiply
  by exp(bias) mask tile, AV matmul (fused denominator), reciprocal +
  broadcast normalize, PE transpose to [hc,t], out-projection, per-wp
  output DMA.
- PSUM banks: qk/v projections share a 2-buf [128,512] tag, dots use two
  2-bank tiles, av/transpose/out-proj share a 2-buf small tag (8 banks
  total).  Split x DMAs ([128,256] quarters) shorten prefetch latency.
  Copies routed via nc.any so the Tile scheduler balances ACT/DVE; the
  exp(bias) mask multiply is split 3 GpSimd / 1 DVE.

Perf notes (measured on trn2 via NTFF; see hwtime.py for the ctypes
axon_start_nrt_profile harness): 294us -> 197us (prior session) ->
~164us (this session).  The big win this session was SOFTWARE
PIPELINING the strips: every engine queue is strict in-order, so the
per-wp tail chain (AV -> recip/normalize -> PE transpose -> out-proj
-> store) used to head-of-line-block PE for ~600-1200ns at each
PE<->DVE hop (~67us of PE idle + HAM re-throttle).  Each tail is now
split into three sub-phases (tail_av / tail_tp / tail_op) and
interleaved at fixed slots between the matmul groups of the NEXT
strip's front phase, so every dependent PE op reaches the queue head
with its cross-engine producer long finished (197 -> 167us).  Smaller
wins: batched per-strip output DMA + merged x loads + hoisted v3
ones-column memsets (HWDGE descriptor gen is 625ns each, serialized),
wqk-first constant DMA order, phase-grouped drain (-> ~164us).
Measured on HW: every LDWEIGHTS costs ~95-105ns regardless of row
count (no FWL on this toolchain; 1536 LDWs = 150us serialized is the
PE floor), MATMUL issue-to-issue for a stream is ~N/2.4 + ~30ns.
Attempted and rejected THIS session: EDM_DVE=0/2 (176us vs 167),
PIPE_LAG=2 (172), merged full-width mask muls (178), v-proj before
mask (179), transposes as regular matmul-by-identity + f32 tps (sim
likes it, HW ~+6us: the f32 aot CAST copies load DVE, the tighter
engine), bigger edm/v3/fo buf rings (180?! deeper rings hurt),
DPS_BUFS=1 or SMALLPS_BUFS=3/4 (psum is bank-granular, 8 banks full).
Prior session's rejects still stand: DMA-XBAR transpose, per-window
garbage-free dots via column tile_position packing (corrupts; also
would double LDW count which is the HW floor), fp8 (error budget).
"""

import os
import sys
import numpy as np

sys.path.insert(0, "/opt/trn_rl_repo")
os.environ.setdefault("JAX_PLATFORMS", "")

import ml_dtypes

BF16 = ml_dtypes.bfloat16

B, L, H, W, C = 2, 8, 64, 64, 256
WS = 8
NHEADS = 8
CH = 32
N_CORES = 8
IMG = B * L                 # 16 images
IMG_PER_CORE = IMG // N_CORES
T_IMG = H * W               # 4096 tokens per image
STRIP = 512                 # tokens per strip (8 windows)
N_STRIPS = T_IMG // STRIP   # 8
NWP = STRIP // 128          # 4 window pairs per strip

_CACHE = {}


def _relative_indices(ws):
    idx = np.array([[i, j] for i in range(ws) for j in range(ws)])
    rel = idx[None, :, :] - idx[:, None, :] + ws - 1
    return rel


def _build_kernel():
    import concourse.bass as bass
    import concourse.mybir as mybir
    import concourse.tile as tile
    from concourse import bacc

    dt = mybir.dt
    nc = bacc.Bacc("TRN2", target_bir_lowering=False, debug=False)

    xT = nc.dram_tensor("xT", [IMG_PER_CORE, C, T_IMG], dt.bfloat16,
                        kind="ExternalInput").ap()
    wqk = nc.dram_tensor("wqk", [C, 512], dt.bfloat16, kind="ExternalInput").ap()
    wv = nc.dram_tensor("wv", [C, C], dt.bfloat16, kind="ExternalInput").ap()
    wout = nc.dram_tensor("wout", [C, C], dt.bfloat16, kind="ExternalInput").ap()
    ebrep = nc.dram_tensor("ebrep", [128, 2048], dt.bfloat16,
                           kind="ExternalInput").ap()
    ident = nc.dram_tensor("ident", [128, 128], dt.bfloat16,
                           kind="ExternalInput").ap()
    out = nc.dram_tensor("out", [IMG_PER_CORE, T_IMG, C], dt.bfloat16,
                         kind="ExternalOutput").ap()

    EXP = mybir.ActivationFunctionType.Exp

    with tile.TileContext(nc) as tc:
        from contextlib import ExitStack
        with ExitStack() as ctx:
            consts = ctx.enter_context(tc.tile_pool(name="consts", bufs=1))
            xp = ctx.enter_context(tc.tile_pool(name="xp", bufs=3))
            qkp = ctx.enter_context(tc.tile_pool(name="qkp", bufs=8))
            vp = ctx.enter_context(tc.tile_pool(name="vp", bufs=8))
            ep = ctx.enter_context(tc.tile_pool(name="ep", bufs=3))
            anp = ctx.enter_context(tc.tile_pool(name="anp", bufs=3))
            aotp = ctx.enter_context(tc.tile_pool(name="aotp", bufs=4))
            rdp = ctx.enter_context(tc.tile_pool(name="rdp", bufs=3))
            fop = ctx.enter_context(tc.tile_pool(name="fop", bufs=3))
            psp = ctx.enter_context(tc.tile_pool(name="psp", bufs=1, space="PSUM"))

            # ---- strip-0 inputs first (head latency), then constants.
            # Big constants are split so no single DMA engine serializes
            # a large transfer in front of the first matmuls.
            pre_x = []
            for half in range(2):
                xt = xp.tile([128, STRIP], dt.bfloat16,
                             tag=("xa" if half == 0 else "xb"), bufs=6)
                for q in range(2):
                    nc.sync.dma_start(
                        out=xt[:, q * 256:(q + 1) * 256],
                        in_=xT[0, half * 128:half * 128 + 128,
                               q * 256:(q + 1) * 256])
                pre_x.append(xt)

            # wqk first (gates the very first matmuls), then wv, then the
            # rest of the constants
            wqk_sb = []
            wv_sb = []
            wout_sb = []
            for kk in range(2):
                wqk_t = consts.tile([128, 512], dt.bfloat16, tag=f"wqk{kk}")
                for q in range(2):
                    nc.sync.dma_start(
                        out=wqk_t[:, q * 256:(q + 1) * 256],
                        in_=wqk[kk * 128:(kk + 1) * 128,
                                q * 256:(q + 1) * 256])
                wqk_sb.append(wqk_t)
            for kk in range(2):
                wv_t = consts.tile([128, 256], dt.bfloat16, tag=f"wv{kk}")
                nc.sync.dma_start(out=wv_t, in_=wv[kk * 128:(kk + 1) * 128, :])
                wv_sb.append(wv_t)
            for kk in range(2):
                wout_t = consts.tile([128, 256], dt.bfloat16, tag=f"wout{kk}")
                nc.sync.dma_start(out=wout_t, in_=wout[kk * 128:(kk + 1) * 128, :])
                wout_sb.append(wout_t)
            eb_sb = consts.tile([128, 4, 512], dt.bfloat16, tag="eb")
            ebr = ebrep.rearrange("p (r c) -> p r c", r=4)
            for r in range(4):
                nc.sync.dma_start(out=eb_sb[:, r, :], in_=ebr[:, r, :])
            id_sb = consts.tile([128, 128], dt.bfloat16, tag="id")
            nc.sync.dma_start(out=id_sb, in_=ident)

            QKPS_BUFS = int(os.environ.get("QKPS_BUFS", "2"))
            SMALLPS_BUFS = int(os.environ.get("SMALLPS_BUFS", "2"))
            DPS_BUFS = int(os.environ.get("DPS_BUFS", "2"))
            EDM_DVE = int(os.environ.get("EDM_DVE", "1"))  # groups on DVE (of 4)
            V3_BUFS = 12
            # ones columns of the v3 ring are written once here; the loop
            # only ever writes [:, :, 0:CH], so they persist across reuse
            for _ in range(V3_BUFS):
                v3i = vp.tile([128, NHEADS, CH + 1], dt.bfloat16,
                              tag="v3", bufs=V3_BUFS)
                nc.gpsimd.memset(v3i[:, :, CH:CH + 1], 1.0)

            orr = out.rearrange("i (s w p) c -> i s p w c",
                                s=N_STRIPS, w=NWP)

            # ---- software pipeline: the per-wp tail chain of strip s-1
            # (AV -> normalize -> transpose -> out-proj -> store) ping-pongs
            # PE<->DVE with no slack, and the in-order PE queue head-of-line
            # blocks on each hop.  Split each tail into three sub-phases and
            # interleave them between the front-phase matmul groups of strip
            # s, so every dependent PE op reaches the queue head with its
            # cross-engine producer long finished.
            def tail_av(st, wp):
                """AV matmuls (+denominator col) and normalize for one wp."""
                edm_sb, v_sb, fo, img_, s_, attn_sb, aot_sbs = st
                cc = (wp % 2) * 128
                avps = psp.tile([128, NHEADS, CH + 1], dt.float32,
                                tag="smallps", bufs=SMALLPS_BUFS)
                for hg in range(2):
                    for rg in range(4):
                        h = 4 * hg + rg
                        nc.tensor.matmul(
                            avps[:, h, :],
                            edm_sb[(hg, rg // 2, wp // 2)][:, rg % 2,
                                                           cc:cc + 128],
                            v_sb[wp][:, h, :],
                            start=True, stop=True,
                        )
                rd = rdp.tile([128, NHEADS, 1], dt.float32, tag="rd", bufs=8)
                nc.vector.reciprocal(rd, avps[:, :, CH:CH + 1])
                attn = anp.tile([128, NHEADS, CH], dt.bfloat16,
                                tag="attn", bufs=8)
                nc.vector.tensor_mul(attn, avps[:, :, 0:CH],
                                     rd.to_broadcast((128, NHEADS, CH)))
                attn_sb[wp] = attn

            def tail_tp(st, wp):
                """transpose [t,hc] -> [hc,t] + evacuate for one wp."""
                edm_sb, v_sb, fo, img_, s_, attn_sb, aot_sbs = st
                attn = attn_sb[wp]
                tps = psp.tile([128, 2, 128], dt.bfloat16,
                               tag="smallps", bufs=SMALLPS_BUFS)
                for half in range(2):
                    nc.tensor.transpose(
                        tps[:, half, :],
                        attn[:, half * 4:(half + 1) * 4, :], id_sb)
                aot = aotp.tile([128, 2, 128], dt.bfloat16, tag="aot", bufs=8)
                nc.any.tensor_copy(aot, tps)
                aot_sbs[wp] = aot

            def tail_op(st, wp):
                """out-projection + store copy for one wp (store DMA on
                the last wp)."""
                edm_sb, v_sb, fo, img_, s_, attn_sb, aot_sbs = st
                aot = aot_sbs[wp]
                ops = psp.tile([128, 256], dt.float32, tag="smallps",
                               bufs=SMALLPS_BUFS)
                nc.tensor.matmul(ops, aot[:, 0, :], wout_sb[0],
                                 start=True, stop=False)
                nc.tensor.matmul(ops, aot[:, 1, :], wout_sb[1],
                                 start=False, stop=True)
                nc.any.tensor_copy(fo[:, wp, :], ops)
                if wp == NWP - 1:
                    nc.sync.dma_start(out=orr[img_, s_], in_=fo)

            PIPE_LAG = int(os.environ.get("PIPE_LAG", "1"))
            pending = []   # oldest-first states awaiting tails
            prev = None
            strips = [(img, s) for img in range(IMG_PER_CORE)
                      for s in range(N_STRIPS)]
            for img, s in strips:
                    prev = pending.pop(0) if len(pending) >= PIPE_LAG else None
                    t0 = s * STRIP
                    if img == 0 and s == 0:
                        xa, xb = pre_x
                    else:
                        xa = xp.tile([128, STRIP], dt.bfloat16, tag="xa", bufs=6)
                        nc.sync.dma_start(out=xa,
                                          in_=xT[img, 0:128, t0:t0 + 512])
                        xb = xp.tile([128, STRIP], dt.bfloat16, tag="xb", bufs=6)
                        nc.sync.dma_start(out=xb,
                                          in_=xT[img, 128:256, t0:t0 + 512])

                    # ---- q/k projection: out [o=128 (4 heads), t=512]
                    # order q03, k03, q47, k47: head-group 0's dots need
                    # only the first two copies
                    qk_sb = [None] * 4
                    for i, ot in enumerate((0, 2, 1, 3)):
                        qkps = psp.tile([128, STRIP], dt.float32, tag="qkps",
                                        bufs=QKPS_BUFS)
                        nc.tensor.matmul(qkps, wqk_sb[0][:, ot * 128:(ot + 1) * 128],
                                         xa, start=True, stop=False)
                        nc.tensor.matmul(qkps, wqk_sb[1][:, ot * 128:(ot + 1) * 128],
                                         xb, start=False, stop=True)
                        qk_t = qkp.tile([128, STRIP], dt.bfloat16, tag="qk_t", bufs=12)
                        nc.any.tensor_copy(qk_t, qkps)
                        qk_sb[ot] = qk_t
                        if prev is not None:
                            if i == 1:
                                tail_av(prev, 0)
                            elif i == 2:
                                tail_av(prev, 1)
                            elif i == 3:
                                tail_tp(prev, 0)

                    # ---- dots -> exp; mask-mult per (hg, half)
                    # v-proj MATMULS are emitted inside the dots loop so
                    # ~1us of PE work separates the last v matmul from the
                    # next strip's first qk matmul (which reuses its qkps
                    # buffer and otherwise stalls ~540ns on the evacuation
                    # copy); the v3 COPIES are emitted after the mask muls
                    # so they don't outrank them in engine priority.
                    edm_sb = {}
                    ed_sb = {}
                    vps_sb = []
                    for j, (hg, half) in enumerate(
                            ((0, 0), (0, 1), (1, 0), (1, 1))):
                            dps = psp.tile([128, 2, 512], dt.float32,
                                           tag="dps", bufs=DPS_BUFS)
                            for wp in range(NWP):
                                c0 = wp * 128
                                for r2 in range(2):
                                    rg = 2 * half + r2
                                    nc.tensor.matmul(
                                        dps[:, r2, c0:c0 + 128],
                                        qk_sb[2 + hg][32 * rg:32 * rg + 32,
                                                      c0:c0 + 128],
                                        qk_sb[hg][32 * rg:32 * rg + 32,
                                                  c0:c0 + 128],
                                        start=True, stop=True,
                                        tile_position=(32 * rg, 0),
                                    )
                            ed = ep.tile([128, 2, 512], dt.bfloat16, tag="ed", bufs=10)
                            nc.scalar.activation(ed, dps, EXP)
                            ed_sb[(hg, half)] = ed
                            if prev is not None:
                                if j == 0:
                                    tail_tp(prev, 1)
                                elif j == 1:
                                    tail_op(prev, 0)
                                    tail_av(prev, 2)
                                elif j == 2:
                                    tail_tp(prev, 2)
                                    tail_op(prev, 1)
                                elif j == 3:
                                    tail_av(prev, 3)
                            if j >= 2:
                                for tb in (0, 1) if j == 2 else (2, 3):
                                    vps = psp.tile([128, NHEADS, CH],
                                                   dt.float32, tag="qkps",
                                                   bufs=QKPS_BUFS)
                                    nc.tensor.matmul(
                                        vps, xa[:, tb * 128:(tb + 1) * 128],
                                        wv_sb[0], start=True, stop=False)
                                    nc.tensor.matmul(
                                        vps, xb[:, tb * 128:(tb + 1) * 128],
                                        wv_sb[1], start=False, stop=True)
                                    vps_sb.append(vps)
                    for wh in range(2):
                        cs = slice(wh * 256, wh * 256 + 256)
                        for hg in range(2):
                            for half in range(2):
                                edm = ep.tile([128, 2, 256], dt.bfloat16,
                                              tag="edm", bufs=24)
                                gi = 2 * hg + half
                                if gi >= 4 - EDM_DVE:
                                    nc.vector.tensor_mul(
                                        edm, ed_sb[(hg, half)][:, :, cs],
                                        eb_sb[:, 2 * half:2 * half + 2, cs])
                                else:
                                    nc.gpsimd.tensor_mul(
                                        edm, ed_sb[(hg, half)][:, :, cs],
                                        eb_sb[:, 2 * half:2 * half + 2, cs])
                                edm_sb[(hg, half, wh)] = edm

                    # v psum evacuation copies (matmuls ran in the dots loop)
                    v_sb = []
                    for tb in range(NWP):
                        v3 = vp.tile([128, NHEADS, CH + 1], dt.bfloat16,
                                     tag="v3", bufs=V3_BUFS)
                        nc.any.tensor_copy(v3[:, :, 0:CH], vps_sb[tb])
                        v_sb.append(v3)
                    if prev is not None:
                        tail_op(prev, 2)
                        tail_tp(prev, 3)
                        tail_op(prev, 3)

                    fo = fop.tile([128, NWP, 256], dt.bfloat16, tag="fo",
                                  bufs=3)
                    pending.append((edm_sb, v_sb, fo, img, s, [None] * NWP,
                                    [None] * NWP))
            # drain remaining strips' tails, phase-grouped so each
            # cross-engine dependency has ~3 window-pairs of slack
            for st in pending:
                for wp in range(NWP):
                    tail_av(st, wp)
                for wp in range(NWP):
                    tail_tp(st, wp)
                for wp in range(NWP):
                    tail_op(st, wp)
    nc.compile()
    return nc


def _host_prep(x, w_qkv, w_out, b_out, pos_embedding):
    ws = WS
    scale = CH ** -0.5
    xs = x.reshape(B * L, H // ws, ws, W // ws, ws, C)
    xs = xs.transpose(0, 1, 3, 2, 4, 5).reshape(IMG, T_IMG, C)
    xT = np.ascontiguousarray(xs.transpose(0, 2, 1)).astype(BF16)

    wq = (w_qkv[:, 0:256] * scale).astype(BF16)
    wk = w_qkv[:, 256:512].astype(BF16)
    wqk = np.concatenate([wq, wk], axis=1)
    wv = w_qkv[:, 512:768].astype(BF16)

    ri = _relative_indices(ws)
    bias = pos_embedding[ri[:, :, 0], ri[:, :, 1]]  # [i, j]
    ebT = np.exp(bias.astype(np.float64)).T.astype(np.float32)  # [j, i]
    ebsuper = np.zeros((128, 128), np.float32)
    ebsuper[0:64, 0:64] = ebT
    ebsuper[64:128, 64:128] = ebT
    ebrep = np.tile(ebsuper, (1, 16)).astype(BF16)

    ident = np.eye(128, dtype=BF16)

    return {
        "xT": xT,
        "wqk": np.ascontiguousarray(wqk),
        "wv": np.ascontiguousarray(wv),
        "wout": w_out.astype(BF16),
        "ebrep": ebrep,
        "ident": ident,
    }


def kernel(x, w_qkv, w_out, b_out, pos_embedding, window_size, **extra):
    from concourse.bass_utils import run_bass_kernel_spmd

    x = np.asarray(x, dtype=np.float32)
    w_qkv = np.asarray(w_qkv, dtype=np.float32)
    w_out = np.asarray(w_out, dtype=np.float32)
    b_out = np.asarray(b_out, dtype=np.float32)
    pos_embedding = np.asarray(pos_embedding, dtype=np.float32)

    prep = _host_prep(x, w_qkv, w_out, b_out, pos_embedding)

    if "nc" not in _CACHE:
        _CACHE["nc"] = _build_kernel()
    nc = _CACHE["nc"]

    in_maps = []
    for core in range(N_CORES):
        m = dict(prep)
        m["xT"] = np.ascontiguousarray(
            prep["xT"][core * IMG_PER_CORE:(core + 1) * IMG_PER_CORE])
        in_maps.append(m)

    res = run_bass_kernel_spmd(nc, in_maps, core_ids=list(range(N_CORES)))
    outs = [res.results[c]["out"] for c in range(N_CORES)]
    o = np.concatenate(outs, axis=0)  # [16, 4096, 256]
    o = o.reshape(B * L, H // WS, W // WS, WS, WS, C)
    o = o.transpose(0, 1, 3, 2, 4, 5).reshape(B, L, H, W, C)
    o = o.astype(np.float32)
    o += b_out.astype(np.float32)
    return np.ascontiguousarray(o)

